# revision 21
# baseline (speedup 1.0000x reference)
"""MoE block kernel for Trainium2 (8 NeuronCores, Bass/Tile).

Strategy: load-balanced expert-parallel slots, bf16 matmuls.
  - Host computes the gate (softmax + top-2) in f64 numpy (0.01% of
    FLOPs) and solves a small covering problem: the 9 FFN jobs (8 routed
    experts + shared expert over all tokens) are split into a UNIFORM
    per-core list of token "slots" (sizes multiple of 64, <= 512), so
    every core executes the identical instruction stream (SPMD) on
    exactly the same token count (~3136 = 24576/8 + padding).
  - Which expert a (core, slot) runs is a pure input-binding choice:
    each slot has its own w1/b1/w2/b2 dram inputs; the host binds any
    expert's (bf16-converted) weights per core.  Pad tokens carry
    scale 0 and are discarded at the scatter.
  - All matmuls are bf16 (stationary + moving) with fp32 PSUM
    accumulation: same PE issue rate as fp32r but half the HBM traffic
    and 2x faster LDWEIGHTS (FWL).  Routing weight and b2 are applied
    at PSUM evacuation: yT = (psum + b2) * wsc, two DVE ops per d-tile.
  - x for slot j+1 is DMA'd at the START of slot j's phase B, before
    the output stores enqueue on the SWDGE ring — the PE never waits
    for activations at slot boundaries (avoids the ~8us idle + ~10us
    HAM K=4/8 re-warm the previous version paid per chunk).

Layouts (per core, per slot of size s<=512):
  phase A: g[i] [128(I), s] = GELU(w1T_i.T @ xT + b1)      (bf16)
  phase B: yT[d] [128(D), s] = (sum_i w2T_(i,d).T @ g[i] + b2_d) * wsc
"""

import itertools
import math
import os

import numpy as np

B, S, D, E, I = 2, 4096, 1024, 8, 4096
T = B * S
TOP_K = 2
P = 128
DT, IT = D // P, I // P          # 8 d-subtiles, 32 i-tiles
UNIT = 32
NSLOT = 7
SIZES = tuple(range(8, 17))      # slot sizes in units (256..512 tokens)
W1G = 4                          # i-tiles per w1 super-tile (1MB DMAs)
W2G = 16                         # i-tiles per w2 group
SHARED = E                       # job id of the shared expert

LAST_RESULTS = None  # BassKernelResults of the most recent run (traced)


# ---------------------------------------------------------------------------
# schedule solver: uniform slot sizes covering the 9 job demands
# ---------------------------------------------------------------------------

def _solve_cover(pool, demands):
    """pool: dict size->count (global). Returns per-demand dict size->count."""
    sizes = sorted(pool, reverse=True)
    order = sorted(range(len(demands)), key=lambda i: -demands[i])
    failed = set()

    def options(avail, d):
        out = []
        maxn = [avail[s] for s in sizes]

        def rec(i, left, pick):
            if left <= 0:
                out.append((-left, dict(pick)))
                return
            if i == len(sizes):
                return
            s = sizes[i]
            if sum(sizes[j] * maxn[j] for j in range(i, len(sizes))) < left:
                return
            hi = min(maxn[i], (left + s - 1) // s + 1)
            for n in range(hi, -1, -1):
                if n * s >= left + 13:
                    continue
                if n:
                    pick[s] = n
                rec(i + 1, left - n * s, pick)
                if n:
                    del pick[s]

        rec(0, d, {})
        out.sort(key=lambda x: x[0])
        return out

    def bt(k, avail):
        if k == len(order):
            return []
        key = (k, tuple(avail[s] for s in sizes))
        if key in failed:
            return None
        for _, pick in options(avail, demands[order[k]]):
            for s, n in pick.items():
                avail[s] -= n
            sub = bt(k + 1, avail)
            for s, n in pick.items():
                avail[s] += n
            if sub is not None:
                return [(order[k], pick)] + sub
        failed.add(key)
        return None

    res = bt(0, dict(pool))
    if res is None:
        return None
    covers = [None] * len(demands)
    for i, pick in res:
        covers[i] = pick
    return covers


def _partitions(u, k, hi):
    """Non-increasing partitions of u into k parts within [SIZES[0], hi]."""
    lo = SIZES[0]
    if k == 1:
        if lo <= u <= hi:
            yield (u,)
        return
    for first in range(min(hi, u - lo * (k - 1)), lo - 1, -1):
        if first * k < u:
            break
        for rest in _partitions(u - first, k - 1, first):
            yield (first,) + rest


def _solve_schedule(counts):
    """counts: 8 routed counts. Returns (slot_sizes_desc_tokens, covers)."""
    demands = [math.ceil(c / UNIT) for c in counts] + [T // UNIT]
    total = sum(demands)
    for U in range(math.ceil(total / 8), math.ceil(total / 8) + 32):
        cands = list(_partitions(U, NSLOT, SIZES[-1]))
        cands.sort(key=lambda ms: (-min(ms), ms))
        for ms in cands:
            pool = {}
            for s in ms:
                pool[s] = pool.get(s, 0) + 8
            covers = _solve_cover(pool, demands)
            if covers is not None:
                sizes = tuple(sorted((s * UNIT for s in ms), reverse=True))
                return sizes, covers
    raise RuntimeError(f"no schedule for counts {counts}")


def _assign_slots(slot_sizes, covers):
    """Map (core, position) -> job id (or None for a dead slot).

    slot_sizes: per-core sizes in tokens, descending. covers: per-job
    dict size_units->count. Returns grid[core][pos] = job | None.
    """
    per_size_entries = {}
    for job, cover in enumerate(covers):
        for su, n in cover.items():
            per_size_entries.setdefault(su * UNIT, []).extend([job] * n)
    grid = [[None] * len(slot_sizes) for _ in range(8)]
    seen = {}
    for pos, s in enumerate(slot_sizes):
        k = seen.get(s, 0)
        seen[s] = k + 1
        entries = per_size_entries.get(s, [])
        for core in range(8):
            idx = k * 8 + core
            grid[core][pos] = entries[idx] if idx < len(entries) else None
    return grid


# ---------------------------------------------------------------------------
# bass program
# ---------------------------------------------------------------------------

def _build_program(slot_sizes):
    import concourse.mybir as mybir
    import concourse.tile as tile
    from concourse import bacc

    F32, BF16 = mybir.dt.float32, mybir.dt.bfloat16
    AF = mybir.ActivationFunctionType

    K = len(slot_sizes)
    CAP = sum(slot_sizes)
    offs = [sum(slot_sizes[:j]) for j in range(K)]

    nc = bacc.Bacc("TRN2", target_bir_lowering=False, debug=False)

    xT_d = nc.dram_tensor("xT", [D, CAP], BF16, kind="ExternalInput")
    wsc_d = nc.dram_tensor("wsc", [P, CAP], F32, kind="ExternalInput")
    w1_d = [nc.dram_tensor(f"w1T_{j}", [D, I], BF16, kind="ExternalInput")
            for j in range(K)]
    b1_d = [nc.dram_tensor(f"b1_{j}", [I], F32, kind="ExternalInput")
            for j in range(K)]
    w2_d = [nc.dram_tensor(f"w2T_{j}", [I, D], BF16, kind="ExternalInput")
            for j in range(K)]
    b2_d = [nc.dram_tensor(f"b2_{j}", [D], F32, kind="ExternalInput")
            for j in range(K)]
    yT_d = nc.dram_tensor("yT", [D, CAP], F32, kind="ExternalOutput")

    xr = xT_d.ap().rearrange("(o p) t -> p o t", p=P)
    outr = yT_d.ap().rearrange("(o p) t -> p o t", p=P)
    w1r = [w.ap().rearrange("(o p) i -> p o i", p=P) for w in w1_d]
    w2r = [w.ap().rearrange("(o p) d -> p o d", p=P) for w in w2_d]

    NSUP = I // (W1G * P)            # 8 w1 super-tiles per slot
    NW2 = IT // W2G                  # 2 w2 groups per d-pair

    with tile.TileContext(nc) as tc:
        with (
            tc.tile_pool(name="const", bufs=1) as const,
            tc.tile_pool(name="act", bufs=1) as act,
            tc.tile_pool(name="xin", bufs=2) as xin,
            tc.tile_pool(name="w1p", bufs=6) as w1p,
            tc.tile_pool(name="w2p", bufs=6) as w2p,
            tc.tile_pool(name="ev", bufs=4) as ev,
            tc.tile_pool(name="psA", bufs=4, space="PSUM") as psA,
            tc.tile_pool(name="psB", bufs=4, space="PSUM") as psB,
        ):
            # --- slot-0 x first on the SWDGE ring, then consts ---
            xts = [None] * K
            def load_x(j):
                xts[j] = xin.tile([P, DT, 512], BF16, tag="x", name="xt")
                s = slot_sizes[j]
                if j == 0:
                    # per-k so the first matmul waits only on subtile 0
                    for k in range(DT):
                        nc.gpsimd.dma_start(xts[j][:, k, :s],
                                            xr[:, k, offs[j]:offs[j] + s])
                else:
                    nc.gpsimd.dma_start(xts[j][:, :, :s],
                                        xr[:, :, offs[j]:offs[j] + s])
            load_x(0)

            # slot-0 biases now; the rest + wsc are issued at slot-0 phase B
            # so they don't sit ahead of cold-start weight DMAs on SWDGE
            b1t = [const.tile([P, IT], F32, tag=f"b1_{j}", name="b1t")
                   for j in range(K)]
            b2t = [const.tile([P, DT], F32, tag=f"b2_{j}", name="b2t")
                   for j in range(K)]
            wsct = const.tile([P, CAP], F32, tag="wsc")

            def load_consts(j):
                nc.gpsimd.dma_start(b1t[j][:], b1_d[j].ap().rearrange(
                    "(o p) -> p o", p=P))
                nc.gpsimd.dma_start(b2t[j][:], b2_d[j].ap().rearrange(
                    "(o p) -> p o", p=P))
            load_consts(0)

            g = act.tile([P, IT, 512], BF16, tag="g")

            w1_pref = {}   # (slot, si) -> tile
            w2_pref = {}   # (slot, dpair, grp) -> tile

            def fetch_w1(j, si, eng):
                t = w1p.tile([P, DT, W1G * P], BF16, tag="w1")
                eng.dma_start(t[:], w1r[j][:, :, si * W1G * P:
                                           (si + 1) * W1G * P])
                return t

            def fetch_w2(j, dp, grp, eng):
                t = w2p.tile([P, W2G, 2 * P], BF16, tag="w2")
                eng.dma_start(t[:], w2r[j][:, grp * W2G:(grp + 1) * W2G,
                                           dp * 2 * P:(dp + 1) * 2 * P])
                return t

            for j in range(K):
                s = slot_sizes[j]
                xt = xts[j]
                # --- phase A ---
                for si in range(NSUP):
                    w1t = w1_pref.pop((j, si), None)
                    if w1t is None and j == 0 and si < 2:
                        # cold start: per-k fetches so the PE can begin on
                        # k=0 after 128KB instead of a full 1MB super-tile
                        eng = nc.sync if si == 0 else nc.scalar
                        w1t = w1p.tile([P, DT, W1G * P], BF16, tag="w1",
                                       name="w1t")
                        for k in range(DT):
                            eng.dma_start(
                                w1t[:, k, :],
                                w1r[j][:, k, si * W1G * P:(si + 1) * W1G * P])
                    elif w1t is None:
                        if j == 0:
                            eng = (nc.gpsimd if si == 2 else
                                   nc.sync if si % 2 == 0 else nc.scalar)
                        else:
                            eng = nc.sync if si % 2 == 0 else nc.scalar
                        w1t = fetch_w1(j, si, eng)
                    if j == 0 and si == 3:
                        nc.gpsimd.dma_start(wsct[:], wsc_d.ap())
                    for sub in range(W1G):
                        i = si * W1G + sub
                        pa = psA.tile([P, 512], F32, tag="psA")
                        for k in range(DT):
                            nc.tensor.matmul(
                                pa[:, :s],
                                w1t[:, k, sub * P:(sub + 1) * P],
                                xt[:, k, :s],
                                start=(k == 0), stop=(k == DT - 1))
                        nc.scalar.activation(g[:, i, :s], pa[:, :s],
                                             AF.Gelu,
                                             bias=b1t[j][:, i, None])
                        # hoist phase-B w2 loads between GELUs.  On the cold
                        # slot the HWDGE rings are saturated with w1, so the
                        # d01/d23 pairs spread over all three queues and go
                        # extra early (the d23 pair was arriving ~7us late).
                        if j == 0:
                            if i == 12:
                                w2_pref[(j, 0, 0)] = fetch_w2(
                                    j, 0, 0, nc.gpsimd)
                            elif i == 16:
                                w2_pref[(j, 0, 1)] = fetch_w2(
                                    j, 0, 1, nc.gpsimd)
                            elif i == 24:
                                w2_pref[(j, 1, 0)] = fetch_w2(j, 1, 0, nc.sync)
                            elif i == 28:
                                w2_pref[(j, 1, 1)] = fetch_w2(
                                    j, 1, 1, nc.scalar)
                        else:
                            if i == 12:
                                w2_pref[(j, 0, 0)] = fetch_w2(
                                    j, 0, 0, nc.scalar)
                            elif i == 20:
                                w2_pref[(j, 0, 1)] = fetch_w2(
                                    j, 0, 1, nc.scalar)
                # --- phase B ---
                if j + 1 < K:
                    load_x(j + 1)   # before out-stores enqueue on SWDGE
                if j == 0:
                    for jj in range(1, K):
                        load_consts(jj)
                w2_cur = None
                for d in range(DT):
                    dp = d // 2
                    if d % 2 == 0:
                        w2_cur = [w2_pref.pop((j, dp, 0)),
                                  w2_pref.pop((j, dp, 1))]
                        # slot 0 fetched d01+d23 in phase A: stay 2 ahead
                        np_ = dp + 2 if j == 0 else dp + 1
                        if np_ < DT // 2 and (j, np_, 0) not in w2_pref:
                            w2_pref[(j, np_, 0)] = fetch_w2(
                                j, np_, 0, nc.sync)
                            w2_pref[(j, np_, 1)] = fetch_w2(
                                j, np_, 1, nc.scalar)
                    if j + 1 < K and d in (2, 4):
                        si = (d - 2) // 2
                        w1_pref[(j + 1, si)] = fetch_w1(
                            j + 1, si, nc.sync if d == 2 else nc.scalar)
                    pb = psB.tile([P, 512], F32, tag="psB")
                    for grp in range(NW2):
                        w2t = w2_cur[grp]
                        for ii in range(W2G):
                            i = grp * W2G + ii
                            nc.tensor.matmul(
                                pb[:, :s],
                                w2t[:, ii, (d % 2) * P:(d % 2 + 1) * P],
                                g[:, i, :s],
                                start=(i == 0), stop=(i == IT - 1))
                    yt = ev.tile([P, 512], F32, tag="ev")
                    nc.vector.tensor_scalar_add(yt[:, :s], pb[:, :s],
                                                b2t[j][:, d, None])
                    nc.vector.tensor_mul(
                        out=yt[:, :s], in0=yt[:, :s],
                        in1=wsct[:, offs[j]:offs[j] + s])
                    # final slot: drain the last stores on the (by then idle)
                    # HWDGE rings instead of queueing behind SWDGE
                    if j == K - 1 and d >= 4:
                        seng = nc.sync if d % 2 == 0 else nc.scalar
                    else:
                        seng = nc.gpsimd
                    seng.dma_start(outr[:, d, offs[j]:offs[j] + s],
                                   yt[:, :s])
                xts[j] = None

    nc.compile()
    return nc


_PROGRAM_CACHE = {}


def _get_program(slot_sizes):
    if slot_sizes not in _PROGRAM_CACHE:
        _PROGRAM_CACHE[slot_sizes] = _build_program(slot_sizes)
    return _PROGRAM_CACHE[slot_sizes]


# ---------------------------------------------------------------------------
# axon trace shim (profiling support under run_bass_kernel_spmd(trace=True))
# ---------------------------------------------------------------------------

def _install_trace_shim():
    import contextlib
    import ctypes
    import sys
    import types

    if "antenv.axon_hooks" in sys.modules:
        return
    so_path = "/opt/axon/libaxon_pjrt.so"
    hook = None
    try:
        lib = ctypes.CDLL(so_path)
        if hasattr(lib, "axon_start_nrt_profile"):
            lib.axon_start_nrt_profile.argtypes = [
                ctypes.POINTER(ctypes.c_int64), ctypes.c_size_t]
            lib.axon_start_nrt_profile.restype = ctypes.c_int64
            lib.axon_stop_nrt_profile.argtypes = [ctypes.c_char_p]
            lib.axon_stop_nrt_profile.restype = ctypes.c_int64

            @contextlib.contextmanager
            def _hook(output_dir, device_ids):
                import jax
                jax.devices()
                if device_ids:
                    ids = (ctypes.c_int64 * len(device_ids))(*device_ids)
                    rc = lib.axon_start_nrt_profile(ids, len(device_ids))
                else:
                    rc = lib.axon_start_nrt_profile(None, 0)
                if rc != 0:
                    raise RuntimeError(f"axon_start_nrt_profile rc={rc}")
                try:
                    yield
                finally:
                    n = lib.axon_stop_nrt_profile(str(output_dir).encode())
                    print(f"ntff profile: {n} file(s) -> {output_dir}",
                          file=sys.stderr)

            hook = _hook
    except OSError:
        pass
    mod = types.ModuleType("antenv.axon_hooks")
    mod.get_axon_ntff_profile_hook = lambda: hook
    mod.set_axon_ntff_profile_hook = lambda h: None
    sys.modules["antenv.axon_hooks"] = mod
    import antenv
    antenv.axon_hooks = mod


# ---------------------------------------------------------------------------
# host dispatch
# ---------------------------------------------------------------------------

def kernel(hidden_states, gate_w, e_w1, e_b1, e_w2, e_b2,
           s_w1, s_b1, s_w2, s_b2):
    global LAST_RESULTS
    import ml_dtypes
    from concourse.bass_utils import run_bass_kernel_spmd

    BF = ml_dtypes.bfloat16
    hidden_states = np.asarray(hidden_states, dtype=np.float32)
    gate_w = np.asarray(gate_w, dtype=np.float32)
    x = np.ascontiguousarray(hidden_states.reshape(T, D))

    # ---- gate: softmax + top-2 (host; 0.01% of total FLOPs) ----
    # f64 so the ranking agrees with any fp32 reference implementation.
    logits = x.astype(np.float64) @ gate_w.T.astype(np.float64)
    m = logits.max(axis=-1, keepdims=True)
    p = np.exp(logits - m)
    p /= p.sum(axis=-1, keepdims=True)
    order = np.argsort(-p, axis=-1, kind="stable")
    top_idx = order[:, :TOP_K]                       # [T, 2]
    top_w = np.take_along_axis(p, top_idx, axis=-1)  # [T, 2]

    idx_e = [np.where((top_idx == e).any(axis=1))[0] for e in range(E)]
    w_e = []
    for e in range(E):
        sel = top_idx[idx_e[e]] == e
        w_e.append((top_w[idx_e[e]] * sel).sum(axis=1).astype(np.float32))
    counts = [len(ix) for ix in idx_e]

    slot_sizes, covers = _solve_schedule(counts)
    grid = _assign_slots(slot_sizes, covers)
    K = len(slot_sizes)
    CAP = sum(slot_sizes)
    offs = [sum(slot_sizes[:j]) for j in range(K)]

    nc = _get_program(slot_sizes)

    # ---- weight conversion (bf16, transposed) ----
    W1T = [np.ascontiguousarray(e_w1[e].T).astype(BF) for e in range(E)]
    W1T.append(np.ascontiguousarray(np.asarray(s_w1, np.float32).T).astype(BF))
    B1 = [np.ascontiguousarray(e_b1[e], dtype=np.float32) for e in range(E)]
    B1.append(np.ascontiguousarray(np.asarray(s_b1, np.float32)))
    W2T = [np.ascontiguousarray(e_w2[e].T).astype(BF) for e in range(E)]
    W2T.append(np.ascontiguousarray(np.asarray(s_w2, np.float32).T).astype(BF))
    B2 = [np.ascontiguousarray(e_b2[e], dtype=np.float32) for e in range(E)]
    B2.append(np.ascontiguousarray(np.asarray(s_b2, np.float32)))

    xT = np.ascontiguousarray(x.T).astype(BF)        # [D, T] bf16

    # ---- per-(core,slot) token ranges: walk each job's slots in order ----
    job_cursor = [0] * (E + 1)
    slot_tok = {}           # (core, pos) -> (job, start, nreal)
    for pos in range(K):
        for core in range(8):
            job = grid[core][pos]
            if job is None:
                continue
            tot = counts[job] if job < E else T
            a = job_cursor[job]
            n = max(0, min(slot_sizes[pos], tot - a))
            job_cursor[job] = a + n
            slot_tok[(core, pos)] = (job, a, n)
    for job in range(E + 1):
        tot = counts[job] if job < E else T
        assert job_cursor[job] >= tot, (job, job_cursor[job], tot)

    in_maps = []
    for core in range(8):
        xcat = np.zeros((D, CAP), BF)
        wsc = np.zeros((CAP,), np.float32)
        im = {"xT": xcat}
        for pos in range(K):
            job, a, n = slot_tok.get((core, pos), (0, 0, 0))
            if n > 0:
                if job < E:
                    tok = idx_e[job][a:a + n]
                    xcat[:, offs[pos]:offs[pos] + n] = xT[:, tok]
                    wsc[offs[pos]:offs[pos] + n] = w_e[job][a:a + n]
                else:
                    xcat[:, offs[pos]:offs[pos] + n] = xT[:, a:a + n]
                    wsc[offs[pos]:offs[pos] + n] = 1.0
            im[f"w1T_{pos}"] = W1T[job]
            im[f"b1_{pos}"] = B1[job]
            im[f"w2T_{pos}"] = W2T[job]
            im[f"b2_{pos}"] = B2[job]
        im["wsc"] = np.ascontiguousarray(
            np.broadcast_to(wsc, (P, CAP)).astype(np.float32))
        in_maps.append(im)

    trace = os.environ.get("MOE_TRACE", "0") == "1"
    kwargs = {}
    if trace:
        _install_trace_shim()
        kwargs = dict(trace=True,
                      tmpdir=os.environ.get("MOE_TRACE_DIR") or None)
    res = run_bass_kernel_spmd(nc, in_maps, core_ids=list(range(8)), **kwargs)
    LAST_RESULTS = res

    y = np.zeros((T, D), np.float32)
    for core in range(8):
        yT = res.results[core]["yT"]
        for pos in range(K):
            job, a, n = slot_tok.get((core, pos), (0, 0, 0))
            if n <= 0:
                continue
            blk = yT[:, offs[pos]:offs[pos] + n].T
            if job < E:
                y[idx_e[job][a:a + n]] += blk
            else:
                y[a:a + n] += blk
    return y.reshape(B, S, D)
